# revision 6
# baseline (speedup 1.0000x reference)
"""ECG spiking encoder (conv-tokenizer + 2x {linear, parametric-LIF} + time-mean)
as a Bass kernel on 8 TRN2 NeuronCores, pure data parallel over batch.

Math (per core, batch shard of 64):
  patches = im2col(x)                       # stride==kernel -> pure relayout
  u1      = sig1*(patches @ Wc.T + bc)      # conv+fc1+sig1 folded on host
  LIF1    : v <- a1*v + u1 ; s = H(v-1) ; soft reset
  u2      = fc2(s1)*sig2 ...                # sign-encoded s1, fold on host
  LIF2    ; out = mean_t(s2)

Engine mapping (per ~1.5us block of 8 time steps):
  DMA    : x as fp8e4 (1 B/elem), two rects per row tile
  PE     : GEMM1 = fp8 DoubleRow 2-pass (W hi/lo, x hi only, W scaled x16,
           descaled in the epilogue); GEMM2 = one bf16 matmul on sign spikes
  Act    : u1/u2 PSUM->SBUF epilogues (bias+scale), L1 spike extract via Sign
  DVE    : the serial LIF scan, one fused custom op per step (both layers
           merged, layer 2 lagging LAG steps), plus the tiny final affine
  Pool   : L2 spike extract (is_ge) + the time-mean reduction
"""
import numpy as np
import ml_dtypes
from contextlib import ExitStack

import concourse.bass as bass
import concourse.tile as tile
from concourse import bacc, mybir
from concourse.bass_utils import run_bass_kernel_spmd

F32 = mybir.dt.float32
BF16 = mybir.dt.bfloat16
FP8 = mybir.dt.float8e4
E4M3 = ml_dtypes.float8_e4m3

# ---- problem constants (hardcoded per contract) ----
B, C, L = 512, 12, 5000
E, H1, H2, P = 128, 128, 128, 50
T = 100
STRIDE = 50
V_TH = 1.0
NCORES = 8
BS = B // NCORES          # 64 batch per core
K = C * P                 # 600 contraction
WSCALE = 16.0             # W1 prescale so the fp8 lo part stays normal-range
LAG = 8                   # layer-2 lag in steps (multiple of 8)
MSTEPS = T + LAG          # 108 merged scan steps
HALF = MSTEPS * BS        # vball column extent of one layer

# 13 row tiles of 8 t-steps (last: 4)
TILES = [(j * 512, 512 if j < 12 else 256) for j in range(13)]
NT = len(TILES)
ROWS = T * BS             # 6400


def _register_lif_op():
    """Fused LIF step as a custom DVE op:
        out = (in0 - (in0 >= s1)) * s0 + in1
    i.e. soft reset of the previous (pre-reset) potential, decay, add input."""
    import concourse.dve_ops as dom
    from concourse.dve_spec import Spec, Src0, Src1, C0, C1, lower, _has_src1
    from concourse.dve_uop import DveOpSpec

    name = "LIF_EMA_RESET_STEP_GE"
    for op in dom.OPS:
        if op.name == name:
            return op

    body = (Src0 - (Src0 >= C1)) * C0 + Src1

    def ref(in0, in1, s0, s1, imm2):
        return (((in0 - (in0 >= s1)) * s0) + in1).astype(np.float32)

    spec = Spec(body=body, reference=ref)
    row = dom._CUSTOM_DVE_ROW_BASE + len(dom.OPS)
    assert row < 0x20
    shas = {}
    for ver in ("v3", "v4"):
        uops = lower(spec, ver=ver)
        shas[ver] = DveOpSpec(name=name, opcode=row, uops=uops,
                              rd1_en=_has_src1(spec)).sha(ver)
    op = dom.DveOp(name, spec, subdim=False, uops_sha=shas)
    dom.OPS.append(op)
    dom._SUB_OPCODE_FOR_NAME[name] = row
    dom.CUSTOM_DVE_SPECS[name] = spec
    return op


def _build_program(a1: float, a2: float):
    lif_op = _register_lif_op()
    nc = bacc.Bacc("TRN2", target_bir_lowering=False, debug=False,
                   num_devices=NCORES)

    XCOLS = sum(6 * ncols for _, ncols in TILES)       # 38400
    x_d = nc.dram_tensor("x8", [128, XCOLS], FP8, kind="ExternalInput").ap()
    w8_d = nc.dram_tensor("w8", [128, 12 * H1], FP8, kind="ExternalInput").ap()
    w2_d = nc.dram_tensor("w2", [128, H2], BF16, kind="ExternalInput").ap()
    b_d = nc.dram_tensor("b1", [128, 3], F32, kind="ExternalInput").ap()
    out_d = nc.dram_tensor("out", [128, BS], F32, kind="ExternalOutput").ap()

    DR = mybir.MatmulPerfMode.DoubleRow
    merged = float(a1) == float(a2)
    assert merged, "non-merged LIF path not implemented (a1 != a2)"

    # per-tile flat x offsets
    flat_offs = []
    off = 0
    for _, ncols in TILES:
        flat_offs.append(off)
        off += 6 * ncols

    with tile.TileContext(nc) as tc, ExitStack() as ctx:
        wpool = ctx.enter_context(tc.tile_pool(name="wpool", bufs=1))
        xpool = ctx.enter_context(tc.tile_pool(name="xpool", bufs=6))
        upool = ctx.enter_context(tc.tile_pool(name="upool", bufs=8))
        spool = ctx.enter_context(tc.tile_pool(name="spool", bufs=3))
        vpool = ctx.enter_context(tc.tile_pool(name="vpool", bufs=1))
        ps1pool = ctx.enter_context(tc.tile_pool(name="ps1", bufs=4, space="PSUM"))
        ps2pool = ctx.enter_context(tc.tile_pool(name="ps2", bufs=2, space="PSUM"))
        mpool = ctx.enter_context(tc.tile_pool(name="mpool", bufs=1))

        # ---- weights / bias ----
        w8 = wpool.tile([128, 12 * H1], FP8)
        nc.gpsimd.dma_start(w8[:], w8_d[:])
        w2t = wpool.tile([128, H2], BF16)
        nc.gpsimd.dma_start(w2t[:], w2_d[:])
        ball = wpool.tile([128, 3], F32)
        nc.gpsimd.dma_start(ball[:], b_d[:])
        b1t = ball[:, 0:1]
        b2t = ball[:, 1:2]
        bneg = ball[:, 2:3]

        # merged v trajectory: L1 at cols [0, HALF), L2 at [HALF, 2*HALF)
        vball = vpool.tile([128, 2 * HALF], F32)
        vb2 = vball[:].rearrange("p (h q) -> p h q", h=2)
        zinit = wpool.tile([128, 128], F32)
        nc.scalar.memzero(zinit[:])

        # layer-2 spike accumulator: acc_t8[s, b] = sum_j s2[8j+s, b]
        acc_t8 = mpool.tile([128, 512], F32)
        nc.scalar.memzero(acc_t8[:])

        # u blocks: [128, 1024] = u1 (8 steps x 64) | u2 (8 steps x 64)
        ublks = [None] * (NT + 1)

        def ublk_for(k):
            if ublks[k] is None:
                ublks[k] = upool.tile([128, 1024], F32, tag="ublk", name=f"ublk{k}")
            return ublks[k]

        m_done = 0

        def emit_scan_through(m_end):
            nonlocal m_done
            while m_done < m_end:
                m = m_done
                ub = ublks[m // 8]
                ub2 = ub[:].rearrange("p (h q) -> p h q", h=2)
                s = m % 8
                src = (zinit[:].rearrange("p (h q) -> p h q", h=2) if m == 0
                       else vb2[:, :, (m - 1) * 64:m * 64])
                nc.vector._custom_dve(
                    lif_op, out=vb2[:, :, m * 64:(m + 1) * 64], in0=src,
                    in1=ub2[:, :, s * 64:(s + 1) * 64], s0=a1, s1=V_TH)
                m_done += 1

        for j in range(NT):
            c0, nc_ = TILES[j]
            fo = flat_offs[j]
            nsteps = nc_ // 64

            # -- x DMA: rectA [128, 4nc], rectB [44, 2nc] --
            xg = xpool.tile([128, 6 * 512], FP8, tag="xg", name=f"xg{j}")
            nc.sync.dma_start(xg[:, 0:4 * nc_], x_d[:, fo:fo + 4 * nc_])
            nc.sync.dma_start(xg[0:44, 4 * nc_:6 * nc_],
                              x_d[0:44, fo + 4 * nc_:fo + 6 * nc_])

            # -- GEMM1: fp8 DoubleRow, 2 passes (W hi, W lo) x 3 k-pairs --
            ps = ps1pool.tile([128, nc_], F32, tag="ps1t", name=f"ps{j}")
            nmm = 0
            for p in range(2):              # hi, lo
                wbase = p * 6 * H1
                for pi, (wo, xo, kp) in enumerate(
                        ((0, 0, 128), (2 * H1, 2 * nc_, 128),
                         (4 * H1, 4 * nc_, 44))):
                    lhsT = w8[0:kp, wbase + wo:wbase + wo + 2 * H1].rearrange(
                        "k (two m) -> k two m", two=2)
                    rhs = xg[0:kp, xo:xo + 2 * nc_].rearrange(
                        "k (two n) -> k two n", two=2)
                    nmm += 1
                    nc.tensor.matmul(ps[:], lhsT, rhs,
                                     start=(nmm == 1), stop=(nmm == 6),
                                     perf_mode=DR)

            # -- u1 epilogue: descale by 1/WSCALE, add bias (Act) --
            ub = ublk_for(j)
            if j == 0:
                nc.scalar.memzero(ub[:, 512:1024])   # u2 of block 0 = 0
            nc.scalar.activation(
                ub[:].rearrange("p (s c) -> p s c", c=64)[:, :nsteps],
                ps[:].rearrange("p (s c) -> p s c", c=64),
                mybir.ActivationFunctionType.Identity, bias=b1t[:, 0:1],
                scale=float(1.0 / WSCALE))
            if j == NT - 1:
                nc.scalar.memzero(ub[:, nsteps * 64:512])
                ubl = ublk_for(NT)
                nc.scalar.memzero(ubl[:, 0:512])     # u1 of tail block = 0

            # -- the serial LIF scan (DVE) --
            emit_scan_through(min(8 * (j + 1), MSTEPS))

            # -- L1 spike extract as sign(v - 1) in {-1,1} (Act), bf16 --
            sb = spool.tile([128, 512], BF16, tag="s1b", name=f"s1b{j}")
            nc.scalar.activation(
                sb[:, :nc_], vball[:, 8 * j * 64:8 * j * 64 + nc_],
                mybir.ActivationFunctionType.Sign, bias=bneg[:, 0:1])

            # -- GEMM2: one bf16 matmul on sign spikes --
            ps2 = ps2pool.tile([128, nc_], F32, tag="ps2t", name=f"ps2{j}")
            nc.tensor.matmul(ps2[:], w2t[:, 0:H2], sb[:, :nc_],
                             start=True, stop=True)

            # -- u2 epilogue into the next block's u2 half (Act) --
            ub_next = ublk_for(j + 1)
            nc.scalar.activation(
                ub_next[:].rearrange("p (s c) -> p s c", c=64)[:, 8:8 + nsteps],
                ps2[:].rearrange("p (s c) -> p s c", c=64),
                mybir.ActivationFunctionType.Identity, bias=b2t[:, 0:1])

            # -- L2 spike extract+accumulate: acc += (v2 >= 1), on Pool --
            if j >= 1:
                r0 = HALF + 8 * j * 64
                nc.vector.scalar_tensor_tensor(
                    acc_t8[:], vball[:, r0:r0 + 512], V_TH, acc_t8[:],
                    mybir.AluOpType.is_ge, mybir.AluOpType.add)

        emit_scan_through(MSTEPS)

        # L2 tail: scan steps [104,108) = t [96,100)
        r0 = HALF + 104 * 64
        nc.vector.scalar_tensor_tensor(
            acc_t8[:, 0:256], vball[:, r0:r0 + 256], V_TH, acc_t8[:, 0:256],
            mybir.AluOpType.is_ge, mybir.AluOpType.add)

        # fold the 8 t-slots (DVE, tiny tail)
        acc = mpool.tile([128, BS], F32, name="accf")
        nc.vector.tensor_reduce(
            acc[:], acc_t8[:].rearrange("p (s b) -> p b s", b=64),
            mybir.AxisListType.X, mybir.AluOpType.add)
        outt = mpool.tile([128, BS], F32, name="outt")
        nc.scalar.activation(outt[:], acc[:],
                             mybir.ActivationFunctionType.Copy,
                             bias=0.0, scale=float(np.float32(1.0 / T)))
        nc.sync.dma_start(out_d[:], outt[:])

    nc.compile()
    return nc


_PROG_CACHE = {}


def get_program(prepped):
    a1, a2 = prepped.a1, prepped.a2
    key = (round(float(a1), 10), round(float(a2), 10))
    if key not in _PROG_CACHE:
        _PROG_CACHE[key] = _build_program(float(a1), float(a2))
    return _PROG_CACHE[key]


class _Prepped:
    __slots__ = ("a1", "a2", "in_maps")


def prepare(x, conv_w, conv_b, fc1_w, fc1_b, fc2_w, fc2_b, w1, w2):
    """Host-side prep: weight folding, im2col relayout, fp8 cast, shards."""
    x = np.asarray(x, np.float32)
    conv_w = np.asarray(conv_w, np.float32)
    conv_b = np.asarray(conv_b, np.float32)
    fc1_w = np.asarray(fc1_w, np.float32)
    fc1_b = np.asarray(fc1_b, np.float32)
    fc2_w = np.asarray(fc2_w, np.float32)
    fc2_b = np.asarray(fc2_b, np.float32)

    sig1 = 1.0 / (1.0 + np.exp(-np.float64(w1)))
    sig2 = 1.0 / (1.0 + np.exp(-np.float64(w2)))
    a1 = np.float32(1.0 - sig1)
    a2 = np.float32(1.0 - sig2)

    # ---- weight folding (fp64 for headroom) ----
    Wc = sig1 * (fc1_w.astype(np.float64) @ conv_w.reshape(E, K).astype(np.float64))
    bc = (sig1 * (fc1_w.astype(np.float64) @ conv_b.astype(np.float64)
                  + fc1_b.astype(np.float64))).astype(np.float32)
    WS = (WSCALE * Wc.T).astype(np.float32)            # [K, H1], prescaled
    Wh = WS.astype(E4M3)
    Wl = (WS - Wh.astype(np.float32)).astype(E4M3)

    def pack_w(Wq):
        p01 = np.concatenate([Wq[0:128], Wq[128:256]], axis=1)      # [128,256]
        p23 = np.concatenate([Wq[256:384], Wq[384:512]], axis=1)
        p4 = np.zeros((128, 2 * H1), E4M3)
        p4[0:44, 0:H1] = Wq[512:556]
        p4[0:44, H1:2 * H1] = Wq[556:600]
        return np.concatenate([p01, p23, p4], axis=1)               # [128,768]

    w8_img = np.concatenate([pack_w(Wh), pack_w(Wl)], axis=1)       # [128,1536]

    # sign-encoded s1 = (g+1)/2:  u2 = (sig2/2)*fc2_w @ g + b2tot
    W2g = (0.5 * sig2 * fc2_w.astype(np.float64))
    b2tot = (sig2 * (0.5 * fc2_w.astype(np.float64).sum(axis=1)
                     + fc2_b.astype(np.float64))).astype(np.float32)
    w2_img = np.ascontiguousarray(W2g.T).astype(ml_dtypes.bfloat16)  # [H1,H2]
    b_img = np.stack([bc, b2tot, np.full(128, -1.0)], axis=1).astype(np.float32)

    # ---- im2col + fp8 + per-tile packing ----
    XCOLS = sum(6 * ncols for _, ncols in TILES)
    in_maps = []
    for ci in range(NCORES):
        xs = x[ci * BS:(ci + 1) * BS].reshape(BS, C, T, P)
        xT = np.ascontiguousarray(xs.transpose(1, 3, 2, 0)).reshape(K, ROWS)
        x8 = xT.astype(E4M3)
        xim = np.zeros((128, XCOLS), E4M3)
        off = 0
        for (c0, ncols) in TILES:
            blk = x8[0:512, c0:c0 + ncols]
            xim[:, off:off + 4 * ncols] = (
                blk.reshape(4, 128, ncols).transpose(1, 0, 2).reshape(128, 4 * ncols))
            blk4 = x8[512:600, c0:c0 + ncols]
            xim[0:44, off + 4 * ncols:off + 6 * ncols] = (
                blk4.reshape(2, 44, ncols).transpose(1, 0, 2).reshape(44, 2 * ncols))
            off += 6 * ncols
        in_maps.append({"x8": xim, "w8": w8_img, "w2": w2_img, "b1": b_img})

    pp = _Prepped()
    pp.a1, pp.a2, pp.in_maps = a1, a2, in_maps
    return pp


def kernel(**inputs):
    pp = prepare(**inputs)
    prog = get_program(pp)
    res = run_bass_kernel_spmd(prog, pp.in_maps, list(range(NCORES)))
    out = np.empty((B, H2), np.float32)
    for ci in range(NCORES):
        out[ci * BS:(ci + 1) * BS] = res.results[ci]["out"].T
    return out


# revision 7
# speedup vs baseline: 1.0943x; 1.0943x over previous
"""ECG spiking encoder (conv-tokenizer + 2x {linear, parametric-LIF} + time-mean)
as a Bass kernel on 8 TRN2 NeuronCores, pure data parallel over batch.

Math (per core, batch shard of 64):
  patches = im2col(x)                       # stride==kernel -> pure relayout
  u1      = sig1*(patches @ Wc.T + bc)      # conv+fc1+sig1 folded on host
  LIF1    : v <- a1*v + u1 ; s = H(v-1) ; soft reset
  u2      = fc2(s1)*sig2 ...                # sign-encoded s1, fold on host
  LIF2    ; out = mean_t(s2)

Engine mapping (per ~1.5us block of 8 time steps):
  DMA    : x as fp8e4 (1 B/elem), two rects per row tile
  PE     : GEMM1 = fp8 DoubleRow 2-pass (W hi/lo, x hi only, W scaled x16,
           descaled in the epilogue); GEMM2 = one bf16 matmul on sign spikes
  Act    : u1/u2 PSUM->SBUF epilogues (bias+scale), L1 spike extract via Sign
  DVE    : the serial LIF scan, one fused custom op per step (both layers
           merged, layer 2 lagging LAG steps), plus the tiny final affine
  Pool   : L2 spike extract (is_ge) + the time-mean reduction
"""
import numpy as np
import ml_dtypes
from contextlib import ExitStack

import concourse.bass as bass
import concourse.tile as tile
from concourse import bacc, mybir
from concourse.bass_utils import run_bass_kernel_spmd

F32 = mybir.dt.float32
BF16 = mybir.dt.bfloat16
FP8 = mybir.dt.float8e4
E4M3 = ml_dtypes.float8_e4m3

# ---- problem constants (hardcoded per contract) ----
B, C, L = 512, 12, 5000
E, H1, H2, P = 128, 128, 128, 50
T = 100
STRIDE = 50
V_TH = 1.0
NCORES = 8
BS = B // NCORES          # 64 batch per core
K = C * P                 # 600 contraction
WSCALE = 16.0             # W1 prescale so the fp8 lo part stays normal-range
LAG = 16                  # layer-2 lag in steps (multiple of 8)
MSTEPS = T + LAG          # 108 merged scan steps
HALF = MSTEPS * BS        # vball column extent of one layer

# 13 row tiles of 8 t-steps (last: 4)
TILES = [(j * 512, 512 if j < 12 else 256) for j in range(13)]
NT = len(TILES)
ROWS = T * BS             # 6400


def _register_lif_op():
    """Fused LIF step as a custom DVE op:
        out = (in0 - (in0 >= s1)) * s0 + in1
    i.e. soft reset of the previous (pre-reset) potential, decay, add input."""
    import concourse.dve_ops as dom
    from concourse.dve_spec import Spec, Src0, Src1, C0, C1, lower, _has_src1
    from concourse.dve_uop import DveOpSpec

    name = "LIF_EMA_RESET_STEP_GE"
    for op in dom.OPS:
        if op.name == name:
            return op

    body = (Src0 - (Src0 >= C1)) * C0 + Src1

    def ref(in0, in1, s0, s1, imm2):
        return (((in0 - (in0 >= s1)) * s0) + in1).astype(np.float32)

    spec = Spec(body=body, reference=ref)
    row = dom._CUSTOM_DVE_ROW_BASE + len(dom.OPS)
    assert row < 0x20
    shas = {}
    for ver in ("v3", "v4"):
        uops = lower(spec, ver=ver)
        shas[ver] = DveOpSpec(name=name, opcode=row, uops=uops,
                              rd1_en=_has_src1(spec)).sha(ver)
    op = dom.DveOp(name, spec, subdim=False, uops_sha=shas)
    dom.OPS.append(op)
    dom._SUB_OPCODE_FOR_NAME[name] = row
    dom.CUSTOM_DVE_SPECS[name] = spec
    return op


def _build_program(a1: float, a2: float):
    lif_op = _register_lif_op()
    nc = bacc.Bacc("TRN2", target_bir_lowering=False, debug=False,
                   num_devices=NCORES)

    XCOLS = sum(6 * ncols for _, ncols in TILES)       # 38400
    x_d = nc.dram_tensor("x8", [128, XCOLS], FP8, kind="ExternalInput").ap()
    w8_d = nc.dram_tensor("w8", [128, 12 * H1], FP8, kind="ExternalInput").ap()
    w2_d = nc.dram_tensor("w2", [128, H2], BF16, kind="ExternalInput").ap()
    b_d = nc.dram_tensor("b1", [128, 3], F32, kind="ExternalInput").ap()
    out_d = nc.dram_tensor("out", [128, BS], F32, kind="ExternalOutput").ap()

    DR = mybir.MatmulPerfMode.DoubleRow
    merged = float(a1) == float(a2)
    assert merged, "non-merged LIF path not implemented (a1 != a2)"

    # per-tile flat x offsets
    flat_offs = []
    off = 0
    for _, ncols in TILES:
        flat_offs.append(off)
        off += 6 * ncols

    with tile.TileContext(nc) as tc, ExitStack() as ctx:
        wpool = ctx.enter_context(tc.tile_pool(name="wpool", bufs=1))
        xpool = ctx.enter_context(tc.tile_pool(name="xpool", bufs=6))
        upool = ctx.enter_context(tc.tile_pool(name="upool", bufs=8))
        spool = ctx.enter_context(tc.tile_pool(name="spool", bufs=3))
        vpool = ctx.enter_context(tc.tile_pool(name="vpool", bufs=1))
        ps1pool = ctx.enter_context(tc.tile_pool(name="ps1", bufs=4, space="PSUM"))
        ps2pool = ctx.enter_context(tc.tile_pool(name="ps2", bufs=2, space="PSUM"))
        mpool = ctx.enter_context(tc.tile_pool(name="mpool", bufs=1))

        # ---- weights / bias ----
        w8 = wpool.tile([128, 12 * H1], FP8)
        nc.gpsimd.dma_start(w8[:], w8_d[:])
        w2t = wpool.tile([128, H2], BF16)
        nc.gpsimd.dma_start(w2t[:], w2_d[:])
        ball = wpool.tile([128, 3], F32)
        nc.gpsimd.dma_start(ball[:], b_d[:])
        b1t = ball[:, 0:1]
        b2t = ball[:, 1:2]
        bneg = ball[:, 2:3]

        # merged v trajectory: L1 at cols [0, HALF), L2 at [HALF, 2*HALF)
        vball = vpool.tile([128, 2 * HALF], F32)
        vb2 = vball[:].rearrange("p (h q) -> p h q", h=2)
        zinit = wpool.tile([128, 128], F32)
        nc.scalar.memzero(zinit[:])

        # layer-2 spike accumulator: acc_t8[s, b] = sum_j s2[8j+s, b]
        acc_t8 = mpool.tile([128, 512], F32)
        nc.scalar.memzero(acc_t8[:])

        # u blocks: [128, 1024] = u1 (8 steps x 64) | u2 (8 steps x 64)
        ublks = [None] * (NT + LAG // 8)

        def ublk_for(k):
            if ublks[k] is None:
                ublks[k] = upool.tile([128, 1024], F32, tag="ublk", name=f"ublk{k}")
            return ublks[k]

        m_done = 0

        def emit_scan_through(m_end):
            nonlocal m_done
            while m_done < m_end:
                m = m_done
                ub = ublks[m // 8]
                ub2 = ub[:].rearrange("p (h q) -> p h q", h=2)
                s = m % 8
                src = (zinit[:].rearrange("p (h q) -> p h q", h=2) if m == 0
                       else vb2[:, :, (m - 1) * 64:m * 64])
                nc.vector._custom_dve(
                    lif_op, out=vb2[:, :, m * 64:(m + 1) * 64], in0=src,
                    in1=ub2[:, :, s * 64:(s + 1) * 64], s0=a1, s1=V_TH)
                m_done += 1

        for j in range(NT):
            c0, nc_ = TILES[j]
            fo = flat_offs[j]
            nsteps = nc_ // 64

            # -- x DMA: rectA [128, 4nc], rectB [44, 2nc] --
            xg = xpool.tile([128, 6 * 512], FP8, tag="xg", name=f"xg{j}")
            nc.sync.dma_start(xg[:, 0:4 * nc_], x_d[:, fo:fo + 4 * nc_])
            nc.sync.dma_start(xg[0:44, 4 * nc_:6 * nc_],
                              x_d[0:44, fo + 4 * nc_:fo + 6 * nc_])

            # -- GEMM1: fp8 DoubleRow, 2 passes (W hi, W lo) x 3 k-pairs --
            ps = ps1pool.tile([128, nc_], F32, tag="ps1t", name=f"ps{j}")
            nmm = 0
            for p in range(2):              # hi, lo
                wbase = p * 6 * H1
                for pi, (wo, xo, kp) in enumerate(
                        ((0, 0, 128), (2 * H1, 2 * nc_, 128),
                         (4 * H1, 4 * nc_, 44))):
                    lhsT = w8[0:kp, wbase + wo:wbase + wo + 2 * H1].rearrange(
                        "k (two m) -> k two m", two=2)
                    rhs = xg[0:kp, xo:xo + 2 * nc_].rearrange(
                        "k (two n) -> k two n", two=2)
                    nmm += 1
                    nc.tensor.matmul(ps[:], lhsT, rhs,
                                     start=(nmm == 1), stop=(nmm == 6),
                                     perf_mode=DR)

            # -- u1 epilogue: descale by 1/WSCALE, add bias (Act) --
            ub = ublk_for(j)
            if j == 0:
                nc.scalar.memzero(ub[:, 512:1024])   # u2 of blocks 0,1 = 0
                ub1 = ublk_for(1)
                nc.scalar.memzero(ub1[:, 512:1024])
            nc.scalar.activation(
                ub[:].rearrange("p (s c) -> p s c", c=64)[:, :nsteps],
                ps[:].rearrange("p (s c) -> p s c", c=64),
                mybir.ActivationFunctionType.Identity, bias=b1t[:, 0:1],
                scale=float(1.0 / WSCALE))
            if j == NT - 1:
                nc.scalar.memzero(ub[:, nsteps * 64:512])
                for k in range(NT, NT + LAG // 8):
                    ubl = ublk_for(k)
                    nc.scalar.memzero(ubl[:, 0:512])     # u1 of tail blocks = 0

            # -- the serial LIF scan (DVE) --
            emit_scan_through(min(8 * (j + 1), MSTEPS))

            # -- L1 spike extract as sign(v - 1) in {-1,1} (Act), bf16 --
            sb = spool.tile([128, 512], BF16, tag="s1b", name=f"s1b{j}")
            nc.scalar.activation(
                sb[:, :nc_], vball[:, 8 * j * 64:8 * j * 64 + nc_],
                mybir.ActivationFunctionType.Sign, bias=bneg[:, 0:1])

            # -- GEMM2: one bf16 matmul on sign spikes --
            ps2 = ps2pool.tile([128, nc_], F32, tag="ps2t", name=f"ps2{j}")
            nc.tensor.matmul(ps2[:], w2t[:, 0:H2], sb[:, :nc_],
                             start=True, stop=True)

            # -- u2 epilogue into the next block's u2 half (Act) --
            ub_next = ublk_for(j + LAG // 8)
            nc.scalar.activation(
                ub_next[:].rearrange("p (s c) -> p s c", c=64)[:, 8:8 + nsteps],
                ps2[:].rearrange("p (s c) -> p s c", c=64),
                mybir.ActivationFunctionType.Identity, bias=b2t[:, 0:1])

            # -- L2 spike extract+accumulate: acc += (v2 >= 1) --
            if j >= LAG // 8:
                r0 = HALF + 8 * j * 64
                nc.vector.scalar_tensor_tensor(
                    acc_t8[:], vball[:, r0:r0 + 512], V_TH, acc_t8[:],
                    mybir.AluOpType.is_ge, mybir.AluOpType.add)

        emit_scan_through(MSTEPS)

        # L2 tail: scan steps [104,116) = t [88,100)
        r0 = HALF + 104 * 64
        nc.vector.scalar_tensor_tensor(
            acc_t8[:], vball[:, r0:r0 + 512], V_TH, acc_t8[:],
            mybir.AluOpType.is_ge, mybir.AluOpType.add)
        r0 = HALF + 112 * 64
        nc.vector.scalar_tensor_tensor(
            acc_t8[:, 0:256], vball[:, r0:r0 + 256], V_TH, acc_t8[:, 0:256],
            mybir.AluOpType.is_ge, mybir.AluOpType.add)

        # fold the 8 t-slots (DVE, tiny tail)
        acc = mpool.tile([128, BS], F32, name="accf")
        nc.vector.tensor_reduce(
            acc[:], acc_t8[:].rearrange("p (s b) -> p b s", b=64),
            mybir.AxisListType.X, mybir.AluOpType.add)
        outt = mpool.tile([128, BS], F32, name="outt")
        nc.scalar.activation(outt[:], acc[:],
                             mybir.ActivationFunctionType.Copy,
                             bias=0.0, scale=float(np.float32(1.0 / T)))
        nc.sync.dma_start(out_d[:], outt[:])

    nc.compile()
    return nc


_PROG_CACHE = {}


def get_program(prepped):
    a1, a2 = prepped.a1, prepped.a2
    key = (round(float(a1), 10), round(float(a2), 10))
    if key not in _PROG_CACHE:
        _PROG_CACHE[key] = _build_program(float(a1), float(a2))
    return _PROG_CACHE[key]


class _Prepped:
    __slots__ = ("a1", "a2", "in_maps")


def prepare(x, conv_w, conv_b, fc1_w, fc1_b, fc2_w, fc2_b, w1, w2):
    """Host-side prep: weight folding, im2col relayout, fp8 cast, shards."""
    x = np.asarray(x, np.float32)
    conv_w = np.asarray(conv_w, np.float32)
    conv_b = np.asarray(conv_b, np.float32)
    fc1_w = np.asarray(fc1_w, np.float32)
    fc1_b = np.asarray(fc1_b, np.float32)
    fc2_w = np.asarray(fc2_w, np.float32)
    fc2_b = np.asarray(fc2_b, np.float32)

    sig1 = 1.0 / (1.0 + np.exp(-np.float64(w1)))
    sig2 = 1.0 / (1.0 + np.exp(-np.float64(w2)))
    a1 = np.float32(1.0 - sig1)
    a2 = np.float32(1.0 - sig2)

    # ---- weight folding (fp64 for headroom) ----
    Wc = sig1 * (fc1_w.astype(np.float64) @ conv_w.reshape(E, K).astype(np.float64))
    bc = (sig1 * (fc1_w.astype(np.float64) @ conv_b.astype(np.float64)
                  + fc1_b.astype(np.float64))).astype(np.float32)
    WS = (WSCALE * Wc.T).astype(np.float32)            # [K, H1], prescaled
    Wh = WS.astype(E4M3)
    Wl = (WS - Wh.astype(np.float32)).astype(E4M3)

    def pack_w(Wq):
        p01 = np.concatenate([Wq[0:128], Wq[128:256]], axis=1)      # [128,256]
        p23 = np.concatenate([Wq[256:384], Wq[384:512]], axis=1)
        p4 = np.zeros((128, 2 * H1), E4M3)
        p4[0:44, 0:H1] = Wq[512:556]
        p4[0:44, H1:2 * H1] = Wq[556:600]
        return np.concatenate([p01, p23, p4], axis=1)               # [128,768]

    w8_img = np.concatenate([pack_w(Wh), pack_w(Wl)], axis=1)       # [128,1536]

    # sign-encoded s1 = (g+1)/2:  u2 = (sig2/2)*fc2_w @ g + b2tot
    W2g = (0.5 * sig2 * fc2_w.astype(np.float64))
    b2tot = (sig2 * (0.5 * fc2_w.astype(np.float64).sum(axis=1)
                     + fc2_b.astype(np.float64))).astype(np.float32)
    w2_img = np.ascontiguousarray(W2g.T).astype(ml_dtypes.bfloat16)  # [H1,H2]
    b_img = np.stack([bc, b2tot, np.full(128, -1.0)], axis=1).astype(np.float32)

    # ---- im2col + fp8 + per-tile packing ----
    XCOLS = sum(6 * ncols for _, ncols in TILES)
    in_maps = []
    for ci in range(NCORES):
        xs = x[ci * BS:(ci + 1) * BS].reshape(BS, C, T, P)
        xT = np.ascontiguousarray(xs.transpose(1, 3, 2, 0)).reshape(K, ROWS)
        x8 = xT.astype(E4M3)
        xim = np.zeros((128, XCOLS), E4M3)
        off = 0
        for (c0, ncols) in TILES:
            blk = x8[0:512, c0:c0 + ncols]
            xim[:, off:off + 4 * ncols] = (
                blk.reshape(4, 128, ncols).transpose(1, 0, 2).reshape(128, 4 * ncols))
            blk4 = x8[512:600, c0:c0 + ncols]
            xim[0:44, off + 4 * ncols:off + 6 * ncols] = (
                blk4.reshape(2, 44, ncols).transpose(1, 0, 2).reshape(44, 2 * ncols))
            off += 6 * ncols
        in_maps.append({"x8": xim, "w8": w8_img, "w2": w2_img, "b1": b_img})

    pp = _Prepped()
    pp.a1, pp.a2, pp.in_maps = a1, a2, in_maps
    return pp


def kernel(**inputs):
    pp = prepare(**inputs)
    prog = get_program(pp)
    res = run_bass_kernel_spmd(prog, pp.in_maps, list(range(NCORES)))
    out = np.empty((B, H2), np.float32)
    for ci in range(NCORES):
        out[ci * BS:(ci + 1) * BS] = res.results[ci]["out"].T
    return out
